# revision 20
# baseline (speedup 1.0000x reference)
"""Trainium2 Bass kernel for tanh-attention (nn_Attention_50362786513376).

reference:
  q = (x @ Wq.T) * dk^-0.5 ; k = x @ Wk.T ; v = x        (heads = 8, dk = 64)
  out = tanh(q k^T) v   per (batch, head),  merged back to [b, n, dim]

Sharding: 8 cores = 4 batches x 2 head-halves (4 heads per core).
Host pre-work (free, exact): transpose x[b] -> xT, slice v channels, slice +
scale + transpose weights. Device per core:
  Q^T = WqT.T @ xT, K^T = WkT.T @ xT     (f16, chunk-chased: projections for
                                          n-chunk t4 start as soon as that
                                          xT column block lands in SBUF)
  per head pair p, i-quarter, j-tile: S^T[j,i] = K^T.T Q^T (row-packed pairs)
  tanh on ScalarE PSUM->SBUF (the throughput bottleneck: n^2*h*b/8 elements)
  out^T[d,i] += v[j,:].T @ tanh(S^T)     (col-tiled pair into one packed
                                          PSUM bank, accumulated over j)
Host post-work: out[b,:,half] = outT.T (f16 staged, upcast on host)
"""
import numpy as np

HEADS = 8
DK = 64
B = 4
N = 2048
DIM = 512
SCALE = DK ** (-0.5)
NCORES = 8
HALF = DIM // 2  # 256 channels per core (4 heads)

_built = None
_built_cfg = None
PROJ_DTYPE = "f16"   # x / weights / projection matmuls
ATTN_DTYPE = "f16"   # Q^T/K^T, qk mms
V_DTYPE = "f16"      # tanh output + v operand of the AV mms
TRACE = False
TRACE_KW = {}
WARM_MM = 9    # dummy matmuls to un-throttle the PE HAM during the DMA wait


def _build():
    from contextlib import ExitStack

    import concourse.tile as tile
    from concourse import bacc, mybir

    F32 = mybir.dt.float32
    DT = {"f32r": mybir.dt.float32r, "f16": mybir.dt.float16,
          "bf16": mybir.dt.bfloat16}
    PROJ_DT = DT[PROJ_DTYPE]
    ATTN_DT = DT[ATTN_DTYPE]
    V_DT = DT[V_DTYPE]
    Tanh = mybir.ActivationFunctionType.Tanh

    nc = bacc.Bacc("TRN2", target_bir_lowering=False, debug=False,
                   num_devices=NCORES)
    xT_ap = nc.dram_tensor("xT", [DIM, N], PROJ_DT, kind="ExternalInput").ap()
    xv_ap = nc.dram_tensor("xv", [N, HALF], V_DT, kind="ExternalInput").ap()
    wqT_ap = nc.dram_tensor("wqT", [DIM, HALF], PROJ_DT,
                            kind="ExternalInput").ap()
    wkT_ap = nc.dram_tensor("wkT", [DIM, HALF], PROJ_DT,
                            kind="ExternalInput").ap()
    outT_ap = nc.dram_tensor("outT", [HALF, N], V_DT,
                             kind="ExternalOutput").ap()

    NT = N // 512          # 4 t-chunks of 512
    NJ = N // 128          # 16 j-tiles

    with tile.TileContext(nc) as tc:
        with ExitStack() as ctx:
            const = ctx.enter_context(tc.tile_pool(name="const", bufs=1))
            qk_pool = ctx.enter_context(tc.tile_pool(name="qk", bufs=1))
            tanh_pool = ctx.enter_context(tc.tile_pool(name="tanh", bufs=6))
            stg_pool = ctx.enter_context(tc.tile_pool(name="stg", bufs=4))

            # ---- input DMAs, chunk-chased ----
            # sync queue: xT n-chunk-major, one 3D DMA per t4 (all 4 ct row
            # blocks); gpsimd queue: weights p0, xv j-chunks, weights p1.
            xT_sb = const.tile([128, 4 * N], PROJ_DT)
            wq_sb = const.tile([128, 4 * HALF], PROJ_DT)
            wk_sb = const.tile([128, 4 * HALF], PROJ_DT)
            xv_sb = const.tile([128, NJ * HALF], V_DT)

            # weights for head-pair 0 first (they gate the very first
            # projection) at the HEAD of the two fast queues
            nc.sync.dma_start(
                wk_sb.rearrange("p (ct c) -> p ct c", ct=4)[:, :, 0:128],
                wkT_ap.rearrange("(ct p) c -> p ct c", p=128)[:, :, 0:128])
            nc.scalar.dma_start(
                wq_sb.rearrange("p (ct c) -> p ct c", ct=4)[:, :, 0:128],
                wqT_ap.rearrange("(ct p) c -> p ct c", p=128)[:, :, 0:128])
            # xT: one DMA per 512-col chunk covering all 4 ct row blocks
            xT3 = xT_sb.rearrange("p (ct n) -> p ct n", ct=4)
            xTsrc = xT_ap.rearrange("(ct p) n -> p ct n", p=128)
            for t4 in range(NT):
                # split each n-chunk across two queues (ct rows 0-1 / 2-3)
                nc.sync.dma_start(xT3[:, 0:2, t4 * 512:(t4 + 1) * 512],
                                  xTsrc[:, 0:2, t4 * 512:(t4 + 1) * 512])
                nc.scalar.dma_start(xT3[:, 2:4, t4 * 512:(t4 + 1) * 512],
                                    xTsrc[:, 2:4, t4 * 512:(t4 + 1) * 512])
            # xv in 4 j-chunks (j-ascending), weights p1 interleaved early
            xv3 = xv_sb.rearrange("p (j c) -> p j c", j=NJ)
            xvsrc = xv_ap.rearrange("(j p) c -> p j c", p=128)
            nc.gpsimd.dma_start(xv3[:, 0:4, :], xvsrc[:, 0:4, :])
            for w_sb, w_ap in ((wk_sb, wkT_ap), (wq_sb, wqT_ap)):
                nc.gpsimd.dma_start(
                    w_sb.rearrange("p (ct c) -> p ct c", ct=4)[:, :, 128:256],
                    w_ap.rearrange("(ct p) c -> p ct c", p=128)[:, :, 128:256])
            for jc in range(1, 4):
                nc.gpsimd.dma_start(xv3[:, 4 * jc:4 * (jc + 1), :],
                                    xvsrc[:, 4 * jc:4 * (jc + 1), :])

            # ---- PSUM pools ----
            # ps_S: 3 x [128,1024] (6 banks); ps_acc: 2 x [128,512] (2 banks,
            # both AV parities col-tiled into one bank at rows 0-63 / 64-127)
            QT = [qk_pool.tile([128, N], ATTN_DT, tag=f"qt{p}", name=f"qt{p}")
                  for p in range(2)]
            KT = [qk_pool.tile([128, N], ATTN_DT, tag=f"kt{p}", name=f"kt{p}")
                  for p in range(2)]
            ps_S = ctx.enter_context(
                tc.tile_pool(name="ps_S", bufs=3, space="PSUM"))
            ps_acc = ctx.enter_context(
                tc.tile_pool(name="ps_acc", bufs=1, space="PSUM"))
            ps_proj = ctx.enter_context(
                tc.tile_pool(name="ps_proj", bufs=1, space="PSUM"))

            # ---- PE warm-up: dummy matmuls during the input-DMA wait so
            # HAM un-throttles before the first projection (sized to end
            # roughly when the first xT chunk lands, not after)
            warm_src = const.tile([128, 512], PROJ_DT, name="warm_src")
            nc.vector.memset(warm_src[:], 0)
            warm = ps_proj.tile([128, 512], F32, tag="proj", name="warm")
            for _ in range(WARM_MM):
                nc.tensor.matmul(warm[:], warm_src[:, 0:128],
                                 warm_src[:], start=True, stop=True)

            # ---- projections (interleaved into the attention stream) ----
            # PSUM borrows rotating ps_S slots.
            def proj_chunk(dst, w_sb, p, t4, pool=None):
                # pre-stream projections borrow idle ps_S slots so they
                # pipeline; in-stream ones use the dedicated proj bank
                if pool is None:
                    ps = ps_proj.tile([128, 512], F32, tag="proj",
                                      name="proj_ps")
                else:
                    ps = pool.tile([128, 1024], F32, tag="S",
                                   name="proj_ps")[:, 0:512]
                for ct in range(4):
                    lhsT = w_sb[:, ct * HALF + p * 128:
                                ct * HALF + (p + 1) * 128]
                    rhs = xT_sb[:, ct * N + t4 * 512:ct * N + t4 * 512 + 512]
                    nc.tensor.matmul(ps[:], lhsT, rhs,
                                     start=(ct == 0), stop=(ct == 3))
                nc.vector.tensor_copy(dst[p][:, t4 * 512:(t4 + 1) * 512],
                                      ps[:])

            def attn_tile(p, iq, j, acc):
                i0 = iq * 512
                S = ps_S.tile([128, 1024], F32, tag="S", name="S")
                # row-packed pair: head parity 0 on PE rows 0-63,
                # parity 1 on rows 64-127
                nc.tensor.matmul(
                    S[:, 0:512],
                    KT[p][0:64, j * 128:(j + 1) * 128],
                    QT[p][0:64, i0:i0 + 512],
                    start=True, stop=True, tile_position=(0, 0))
                nc.tensor.matmul(
                    S[:, 512:1024],
                    KT[p][64:128, j * 128:(j + 1) * 128],
                    QT[p][64:128, i0:i0 + 512],
                    start=True, stop=True, tile_position=(64, 0))
                T = tanh_pool.tile([128, 1024], V_DT, tag="T", name="T")
                nc.scalar.activation(T[:], S[:], Tanh)
                # AV pair col-tiled: par0 -> acc rows 0-63 (PE cols 0-63),
                # par1 -> acc rows 64-127 (PE cols 64-127); concurrent on
                # disjoint col groups
                for par in range(2):
                    lh = 2 * p + par
                    v = xv_sb[:, j * HALF + lh * 64:j * HALF + lh * 64 + 64]
                    nc.tensor.matmul(
                        acc[par * 64:(par + 1) * 64, :],
                        v,
                        T[:, par * 512:(par + 1) * 512],
                        start=(j == 0), stop=(j == NJ - 1),
                        tile_position=(0, par * 64))

            def store_acc(p, iq, acc):
                st = stg_pool.tile([128, 512], V_DT, tag="stg", name="stg")
                nc.vector.tensor_copy(st[:], acc[:])
                nc.gpsimd.dma_start(
                    outT_ap[p * 128:(p + 1) * 128, iq * 512:(iq + 1) * 512],
                    st[:])

            # ---- stream schedule ----
            # (p0, iq0): j-tiles chase the xT chunks; KT0 chunk t4 is
            # projected right before the j-tiles that read it (j = 4*t4 ..).
            proj_chunk(KT, wk_sb, 0, 0, pool=ps_S)
            proj_chunk(QT, wq_sb, 0, 0, pool=ps_S)
            acc = ps_acc.tile([128, 512], F32, tag="acc", name="acc")
            for t4 in range(NT):
                if t4 > 0:
                    proj_chunk(KT, wk_sb, 0, t4)
                for j in range(4 * t4, 4 * t4 + 4):
                    attn_tile(0, 0, j, acc)
                    if j == 13:
                        proj_chunk(QT, wq_sb, 0, 1)
            store_acc(0, 0, acc)
            # (p0, iq 1..3): QT0 chunk for the NEXT iq projected just-in-time
            # at j=13 of the previous stream; p1 projections spread at
            # j=3,8,13 (TensorE slack, dedicated proj PSUM bank).
            p1_proj = [(KT, wk_sb, 1, 0), (QT, wq_sb, 1, 0),
                       (KT, wk_sb, 1, 1), (QT, wq_sb, 1, 1),
                       (KT, wk_sb, 1, 2), (QT, wq_sb, 1, 2),
                       (KT, wk_sb, 1, 3), (QT, wq_sb, 1, 3)]
            for iq in range(1, 4):
                acc = ps_acc.tile([128, 512], F32, tag="acc", name="acc")
                for j in range(NJ):
                    attn_tile(0, iq, j, acc)
                    if j == 13 and iq < 3:
                        proj_chunk(QT, wq_sb, 0, iq + 1)
                    elif j in (3, 8) and p1_proj:
                        proj_chunk(*p1_proj.pop(0))
                    elif j == 13 and iq == 3 and p1_proj:
                        proj_chunk(*p1_proj.pop(0))
                store_acc(0, iq, acc)
            # (p1, iq 0..3)
            for iq in range(4):
                acc = ps_acc.tile([128, 512], F32, tag="acc", name="acc")
                for j in range(NJ):
                    attn_tile(1, iq, j, acc)
                    if j in (3, 8, 13) and p1_proj:
                        proj_chunk(*p1_proj.pop(0))
                store_acc(1, iq, acc)

    nc.compile()
    return nc


def _get_built():
    global _built, _built_cfg
    cfg = (PROJ_DTYPE, ATTN_DTYPE, V_DTYPE)
    if _built is None or _built_cfg != cfg:
        _built = _build()
        _built_cfg = cfg
    return _built


def kernel(x, Wq, Wk):
    from concourse.bass_utils import run_bass_kernel_spmd

    x = np.asarray(x, dtype=np.float32)
    Wq = np.asarray(Wq, dtype=np.float32)
    Wk = np.asarray(Wk, dtype=np.float32)

    proj_np = np.float16
    v_np = np.float16

    nc = _get_built()
    in_maps = []
    for c in range(NCORES):
        b, half = c // 2, c % 2
        sl = slice(half * HALF, (half + 1) * HALF)
        in_maps.append({
            "xT": np.ascontiguousarray(x[b].T).astype(proj_np),
            "xv": np.ascontiguousarray(x[b][:, sl]).astype(v_np),
            "wqT": np.ascontiguousarray((SCALE * Wq[sl, :]).T).astype(proj_np),
            "wkT": np.ascontiguousarray(Wk[sl, :].T).astype(proj_np),
        })
    try:
        res = run_bass_kernel_spmd(nc, in_maps, core_ids=list(range(NCORES)),
                                   trace=TRACE, **TRACE_KW)
    except Exception:
        # transient device wedge (NRT_EXEC_UNIT_UNRECOVERABLE) recovers on
        # retry; one attempt is enough in practice
        import time as _time
        _time.sleep(2.0)
        res = run_bass_kernel_spmd(nc, in_maps, core_ids=list(range(NCORES)),
                                   trace=TRACE, **TRACE_KW)
    out = np.empty((B, N, DIM), np.float32)
    for c in range(NCORES):
        b, half = c // 2, c % 2
        out[b, :, half * HALF:(half + 1) * HALF] = \
            res.results[c]["outT"].T.astype(np.float32)
    if TRACE:
        kernel.last_results = res
    return out


# revision 30
# speedup vs baseline: 1.0247x; 1.0247x over previous
"""Trainium2 Bass kernel for tanh-attention (nn_Attention_50362786513376).

reference:
  q = (x @ Wq.T) * dk^-0.5 ; k = x @ Wk.T ; v = x        (heads = 8, dk = 64)
  out = tanh(q k^T) v   per (batch, head),  merged back to [b, n, dim]

Sharding: 8 cores = 4 batches x 2 head-halves (4 heads per core).
Host pre-work (free, exact): transpose x[b] -> xT, slice v channels, slice +
scale + transpose weights. Device per core:
  Q^T = WqT.T @ xT, K^T = WkT.T @ xT     (f16, chunk-chased: projections for
                                          n-chunk t4 start as soon as that
                                          xT column block lands in SBUF)
  per head pair p, i-quarter, j-tile: S^T[j,i] = K^T.T Q^T (row-packed pairs)
  tanh on ScalarE PSUM->SBUF (the throughput bottleneck: n^2*h*b/8 elements)
  out^T[d,i] += v[j,:].T @ tanh(S^T)     (col-tiled pair into one packed
                                          PSUM bank, accumulated over j)
Host post-work: out[b,:,half] = outT.T (f16 staged, upcast on host)
"""
import numpy as np

HEADS = 8
DK = 64
B = 4
N = 2048
DIM = 512
SCALE = DK ** (-0.5)
NCORES = 8
HALF = DIM // 2  # 256 channels per core (4 heads)

_built = None
_built_cfg = None
PROJ_DTYPE = "f16"   # x / weights / projection matmuls
ATTN_DTYPE = "f16"   # Q^T/K^T, qk mms
V_DTYPE = "f16"      # tanh output + v operand of the AV mms
TRACE = False
TRACE_KW = {}
WARM_MM = 9    # dummy matmuls to un-throttle the PE HAM during the DMA wait


def _build():
    from contextlib import ExitStack

    import concourse.tile as tile
    from concourse import bacc, mybir

    F32 = mybir.dt.float32
    DT = {"f32r": mybir.dt.float32r, "f16": mybir.dt.float16,
          "bf16": mybir.dt.bfloat16}
    PROJ_DT = DT[PROJ_DTYPE]
    ATTN_DT = DT[ATTN_DTYPE]
    V_DT = DT[V_DTYPE]
    Tanh = mybir.ActivationFunctionType.Tanh

    nc = bacc.Bacc("TRN2", target_bir_lowering=False, debug=False,
                   num_devices=NCORES)
    # All inputs are host-packed into the exact SBUF images so every DMA is
    # contiguous 2D with multi-KB lines:
    #   xT_img [128, t4-major (t4, ct, 512)], xv_img [128, (j, 256)],
    #   w*_img [128, p-major (p, ct, 128)]
    xT_ap = nc.dram_tensor("xT", [128, 4 * N], PROJ_DT,
                           kind="ExternalInput").ap()
    xv_ap = nc.dram_tensor("xv", [128, (N // 128) * HALF], V_DT,
                           kind="ExternalInput").ap()
    wqT_ap = nc.dram_tensor("wqT", [128, 4 * HALF], PROJ_DT,
                            kind="ExternalInput").ap()
    wkT_ap = nc.dram_tensor("wkT", [128, 4 * HALF], PROJ_DT,
                            kind="ExternalInput").ap()
    outT_ap = nc.dram_tensor("outT", [HALF, N], V_DT,
                             kind="ExternalOutput").ap()

    NT = N // 512          # 4 t-chunks of 512
    NJ = N // 128          # 16 j-tiles

    with tile.TileContext(nc) as tc:
        with ExitStack() as ctx:
            const = ctx.enter_context(tc.tile_pool(name="const", bufs=1))
            qk_pool = ctx.enter_context(tc.tile_pool(name="qk", bufs=1))
            tanh_pool = ctx.enter_context(tc.tile_pool(name="tanh", bufs=6))
            stg_pool = ctx.enter_context(tc.tile_pool(name="stg", bufs=4))

            # ---- input DMAs: all contiguous 2D copies of host-packed
            # SBUF images, chunk-chased across queues ----
            # SBUF layouts: xT_sb col = t4*2048 + ct*512 + c (t4-major);
            # w_sb col = p*512 + ct*128 + c (p-major); xv_sb col = j*256 + c
            xT_sb = const.tile([128, 4 * N], PROJ_DT)
            wq_sb = const.tile([128, 4 * HALF], PROJ_DT)
            wk_sb = const.tile([128, 4 * HALF], PROJ_DT)
            xv_sb = const.tile([128, NJ * HALF], V_DT)

            # weights for head-pair 0 at the head of the two fast queues,
            # then xT chunks alternating sync/scalar
            nc.sync.dma_start(wk_sb[:, 0:512], wkT_ap[:, 0:512])
            nc.scalar.dma_start(wq_sb[:, 0:512], wqT_ap[:, 0:512])
            for t4 in range(NT):
                q = nc.sync if t4 % 2 == 0 else nc.scalar
                q.dma_start(xT_sb[:, t4 * 2048:(t4 + 1) * 2048],
                            xT_ap[:, t4 * 2048:(t4 + 1) * 2048])
            # xv in 4 j-chunks (j-ascending) + p1 weights on the gpsimd queue
            for jc in range(4):
                nc.gpsimd.dma_start(xv_sb[:, jc * 1024:(jc + 1) * 1024],
                                    xv_ap[:, jc * 1024:(jc + 1) * 1024])
                if jc == 0:
                    nc.gpsimd.dma_start(wk_sb[:, 512:1024],
                                        wkT_ap[:, 512:1024])
                    nc.gpsimd.dma_start(wq_sb[:, 512:1024],
                                        wqT_ap[:, 512:1024])

            # ---- PSUM pools ----
            # ps_S: 3 x [128,1024] (6 banks); ps_acc: 2 x [128,512] (2 banks,
            # both AV parities col-tiled into one bank at rows 0-63 / 64-127)
            QT = [qk_pool.tile([128, N], ATTN_DT, tag=f"qt{p}", name=f"qt{p}")
                  for p in range(2)]
            KT = [qk_pool.tile([128, N], ATTN_DT, tag=f"kt{p}", name=f"kt{p}")
                  for p in range(2)]
            ps_S = ctx.enter_context(
                tc.tile_pool(name="ps_S", bufs=3, space="PSUM"))
            ps_acc = ctx.enter_context(
                tc.tile_pool(name="ps_acc", bufs=1, space="PSUM"))
            ps_proj = ctx.enter_context(
                tc.tile_pool(name="ps_proj", bufs=1, space="PSUM"))

            # ---- PE warm-up: dummy matmuls during the input-DMA wait so
            # HAM un-throttles before the first projection (sized to end
            # roughly when the first xT chunk lands, not after)
            warm_src = const.tile([128, 512], PROJ_DT, name="warm_src")
            nc.vector.memset(warm_src[:], 0)
            warm = ps_proj.tile([128, 512], F32, tag="proj", name="warm")
            for _ in range(WARM_MM):
                nc.tensor.matmul(warm[:], warm_src[:, 0:128],
                                 warm_src[:], start=True, stop=True)

            # ---- projections (interleaved into the attention stream) ----
            # PSUM borrows rotating ps_S slots.
            def proj_chunk(dst, w_sb, p, t4, pool=None):
                # pre-stream projections borrow idle ps_S slots so they
                # pipeline; in-stream ones use the dedicated proj bank
                if pool is None:
                    ps = ps_proj.tile([128, 512], F32, tag="proj",
                                      name="proj_ps")
                else:
                    ps = pool.tile([128, 1024], F32, tag="S",
                                   name="proj_ps")[:, 0:512]
                for ct in range(4):
                    lhsT = w_sb[:, p * 512 + ct * 128:p * 512 + ct * 128 + 128]
                    rhs = xT_sb[:, t4 * 2048 + ct * 512:
                                t4 * 2048 + ct * 512 + 512]
                    nc.tensor.matmul(ps[:], lhsT, rhs,
                                     start=(ct == 0), stop=(ct == 3))
                nc.vector.tensor_copy(dst[p][:, t4 * 512:(t4 + 1) * 512],
                                      ps[:])

            def attn_tile(p, iq, j, acc):
                i0 = iq * 512
                S = ps_S.tile([128, 1024], F32, tag="S", name="S")
                # row-packed pair: head parity 0 on PE rows 0-63,
                # parity 1 on rows 64-127
                nc.tensor.matmul(
                    S[:, 0:512],
                    KT[p][0:64, j * 128:(j + 1) * 128],
                    QT[p][0:64, i0:i0 + 512],
                    start=True, stop=True, tile_position=(0, 0))
                nc.tensor.matmul(
                    S[:, 512:1024],
                    KT[p][64:128, j * 128:(j + 1) * 128],
                    QT[p][64:128, i0:i0 + 512],
                    start=True, stop=True, tile_position=(64, 0))
                T = tanh_pool.tile([128, 1024], V_DT, tag="T", name="T")
                nc.scalar.activation(T[:], S[:], Tanh)
                # AV pair col-tiled: par0 -> acc rows 0-63 (PE cols 0-63),
                # par1 -> acc rows 64-127 (PE cols 64-127); concurrent on
                # disjoint col groups
                for par in range(2):
                    lh = 2 * p + par
                    v = xv_sb[:, j * HALF + lh * 64:j * HALF + lh * 64 + 64]
                    nc.tensor.matmul(
                        acc[par * 64:(par + 1) * 64, :],
                        v,
                        T[:, par * 512:(par + 1) * 512],
                        start=(j == 0), stop=(j == NJ - 1),
                        tile_position=(0, par * 64))

            def store_acc(p, iq, acc):
                st = stg_pool.tile([128, 512], V_DT, tag="stg", name="stg")
                nc.vector.tensor_copy(st[:], acc[:])
                nc.gpsimd.dma_start(
                    outT_ap[p * 128:(p + 1) * 128, iq * 512:(iq + 1) * 512],
                    st[:])

            # ---- stream schedule ----
            # (p0, iq0): j-tiles chase the xT chunks; KT0 chunk t4 is
            # projected right before the j-tiles that read it (j = 4*t4 ..).
            proj_chunk(KT, wk_sb, 0, 0, pool=ps_S)
            proj_chunk(QT, wq_sb, 0, 0, pool=ps_S)
            acc = ps_acc.tile([128, 512], F32, tag="acc", name="acc")
            for t4 in range(NT):
                if t4 > 0:
                    proj_chunk(KT, wk_sb, 0, t4)
                for j in range(4 * t4, 4 * t4 + 4):
                    attn_tile(0, 0, j, acc)
                    if j == 13:
                        proj_chunk(QT, wq_sb, 0, 1)
            store_acc(0, 0, acc)
            # (p0, iq 1..3): QT0 chunk for the NEXT iq projected just-in-time
            # at j=13 of the previous stream; p1 projections spread at
            # j=3,8,13 (TensorE slack, dedicated proj PSUM bank).
            p1_proj = [(KT, wk_sb, 1, 0), (QT, wq_sb, 1, 0),
                       (KT, wk_sb, 1, 1), (QT, wq_sb, 1, 1),
                       (KT, wk_sb, 1, 2), (QT, wq_sb, 1, 2),
                       (KT, wk_sb, 1, 3), (QT, wq_sb, 1, 3)]
            for iq in range(1, 4):
                acc = ps_acc.tile([128, 512], F32, tag="acc", name="acc")
                for j in range(NJ):
                    attn_tile(0, iq, j, acc)
                    if j == 13 and iq < 3:
                        proj_chunk(QT, wq_sb, 0, iq + 1)
                    elif j in (3, 8) and p1_proj:
                        proj_chunk(*p1_proj.pop(0))
                    elif j == 13 and iq == 3 and p1_proj:
                        proj_chunk(*p1_proj.pop(0))
                store_acc(0, iq, acc)
            # (p1, iq 0..3)
            for iq in range(4):
                acc = ps_acc.tile([128, 512], F32, tag="acc", name="acc")
                for j in range(NJ):
                    attn_tile(1, iq, j, acc)
                    if j in (3, 8, 13) and p1_proj:
                        proj_chunk(*p1_proj.pop(0))
                store_acc(1, iq, acc)

    nc.compile()
    return nc


def _get_built():
    global _built, _built_cfg
    cfg = (PROJ_DTYPE, ATTN_DTYPE, V_DTYPE)
    if _built is None or _built_cfg != cfg:
        _built = _build()
        _built_cfg = cfg
    return _built


def kernel(x, Wq, Wk):
    from concourse.bass_utils import run_bass_kernel_spmd

    x = np.asarray(x, dtype=np.float32)
    Wq = np.asarray(Wq, dtype=np.float32)
    Wk = np.asarray(Wk, dtype=np.float32)

    proj_np = np.float16
    v_np = np.float16

    def pack_xT(xb):
        # [512, 2048] -> img[part, t4*2048 + ct*512 + c]
        xT = xb.T.astype(proj_np)
        return np.ascontiguousarray(
            xT.reshape(4, 128, 4, 512).transpose(1, 2, 0, 3)
            .reshape(128, 8192))

    def pack_w(wT):
        # [512, 256] -> img[part, p*512 + ct*128 + c]
        return np.ascontiguousarray(
            wT.reshape(4, 128, 2, 128).transpose(1, 2, 0, 3)
            .reshape(128, 1024))

    def pack_xv(xvb):
        # [2048, 256] -> img[part, j*256 + c]
        return np.ascontiguousarray(
            xvb.reshape(16, 128, 256).transpose(1, 0, 2).reshape(128, 4096))

    nc = _get_built()
    in_maps = []
    for c in range(NCORES):
        b, half = c // 2, c % 2
        sl = slice(half * HALF, (half + 1) * HALF)
        in_maps.append({
            "xT": pack_xT(x[b]),
            "xv": pack_xv(x[b][:, sl].astype(v_np)),
            "wqT": pack_w((SCALE * Wq[sl, :]).T.astype(proj_np)),
            "wkT": pack_w(Wk[sl, :].T.astype(proj_np)),
        })
    try:
        res = run_bass_kernel_spmd(nc, in_maps, core_ids=list(range(NCORES)),
                                   trace=TRACE, **TRACE_KW)
    except Exception:
        # transient device wedge (NRT_EXEC_UNIT_UNRECOVERABLE) recovers on
        # retry; one attempt is enough in practice
        import time as _time
        _time.sleep(2.0)
        res = run_bass_kernel_spmd(nc, in_maps, core_ids=list(range(NCORES)),
                                   trace=TRACE, **TRACE_KW)
    out = np.empty((B, N, DIM), np.float32)
    for c in range(NCORES):
        b, half = c // 2, c % 2
        out[b, :, half * HALF:(half + 1) * HALF] = \
            res.results[c]["outT"].T.astype(np.float32)
    if TRACE:
        kernel.last_results = res
    return out
